# revision 23
# baseline (speedup 1.0000x reference)
"""AttentionClustering (vq_codebook) Trainium2 kernel, 8-core data parallel.

Shard: 8 cores = 4 images x 2 half-images (128 output rows each). Odd cores
get a vertically flipped shard + row-flipped conv weights so every core's
program is identical (true image edge at local top, interior halo at bottom).

Math: q1 = relu(conv3x3(x, w1) + b1); q2 = relu(conv3x3(q1, w2) + b2)  (both
with replicate padding); then the 1x1 conv + cluster-distance softmax folds to
  logit[px, k] = sum_ci q2[ci, px] * muW[k, ci] + bp[k]
  muW = 2 * mu @ W3,  bp = 2 * mu @ b3 - |mu|^2      (|q|^2 cancels in softmax)
  out[px] = sum_k softmax_k(logit) * label[k]

All matmuls run in fp16 (values are tiny; measured end-to-end rel err vs the
f32 reference = 1.5e-3): full PE rate at any N, and fp16 LDWEIGHTS (~97ns via
fast-weight-load) hides entirely under the N=512 matmul stream (~216ns/MM).
PE measures >94% busy; HW exec ~762us on 8 cores.
"""
import sys
if '/opt/trn_rl_repo' not in sys.path:
    sys.path.insert(0, '/opt/trn_rl_repo')

import numpy as np
import concourse.bass as bass
import concourse.mybir as mybir
from concourse import bacc, tile
from concourse.bass_utils import run_bass_kernel_spmd

F32 = mybir.dt.float32
F16 = mybir.dt.float16
AF = mybir.ActivationFunctionType
ALU = mybir.AluOpType
AX = mybir.AxisListType

B, CIN, H, W = 4, 64, 256, 256
Q, K = 256, 16
RB = 32           # output rows per band
NBAND = 4         # bands per core (128 rows)
NCORES = 8

_cached = {}


def build_nc():
    nc = bacc.Bacc("TRN2", target_bir_lowering=False, debug=False)

    CHS = 132 * (W + 2)          # per-channel element stride in flat xh
    xh = nc.declare_dram_parameter("xh", [CIN * CHS + 2 * (W + 2)], F16,
                                   isOutput=False)
    w1a = nc.declare_dram_parameter("w1a", [128, 6, 128], F16, isOutput=False)
    w1r = nc.declare_dram_parameter("w1r", [128, 2, 128], F16, isOutput=False)
    w1s = nc.declare_dram_parameter("w1s", [128, 2, 128], F16, isOutput=False)
    w2l = nc.declare_dram_parameter("w2l", [128, 48, 128], F16, isOutput=False)
    muw = nc.declare_dram_parameter("muw", [128, 2, K], F16, isOutput=False)
    cst = nc.declare_dram_parameter("cst", [128, 2 * K + 4], F32, isOutput=False)
    idn = nc.declare_dram_parameter("idn", [128, 128], F32, isOutput=False)
    outd = nc.declare_dram_parameter("out", [128, W], F32, isOutput=True)

    with tile.TileContext(nc) as tc:
        with tc.tile_pool(name="singles", bufs=1) as singles, \
             tc.tile_pool(name="xpool", bufs=2) as xpool, \
             tc.tile_pool(name="q1pool", bufs=1) as q1pool, \
             tc.tile_pool(name="q2pool", bufs=2) as q2pool, \
             tc.tile_pool(name="tpool", bufs=2) as tpool, \
             tc.tile_pool(name="smx", bufs=2) as smx, \
             tc.tile_pool(name="obuf", bufs=2) as obuf, \
             tc.tile_pool(name="ps1", bufs=2, space="PSUM") as ps1, \
             tc.tile_pool(name="ps2", bufs=2, space="PSUM") as ps2, \
             tc.tile_pool(name="psl", bufs=1, space="PSUM") as psl:

            # w1a loads first (small); its landing also unblocks the PE
            # warmup matmuls (junk math into a recycled psum slot) that keep
            # the HAM clock-gate ramping through the initial x DMA wait.
            w1abuf = singles.tile([128, 6, 128], F16, tag="w1abuf")
            nc.sync.dma_start(out=w1abuf, in_=w1a.ap())
            w1a_sb = {(mc, dr): w1abuf[:, mc * 3 + dr, :]
                      for mc in range(2) for dr in range(3)}
            wmv = w1abuf.rearrange("p a b -> p (a b)")[:, 0:512] \
                .rearrange("p (a b) -> p a b", a=2)
            for _ in range(16):
                wps = ps1.tile([128, 2, W], F32, tag="c1ps", name="wps")
                nc.tensor.matmul(wps, w1abuf[:, 0, :], wmv,
                                 start=True, stop=True)

            xh_ap = xh.ap()

            def xsrc(r0, lo, hi, shift):
                # [64ch, (rows cols) flat] slice of xh, shifted by `shift`
                # elements (1 = one column, W+2 = one row). Rows within a
                # channel are contiguous, so flattening (rows, cols) into one
                # dim gives one big descriptor per channel instead of one
                # per row (10x fewer; the startup chunk went from ~28us to
                # ~4us of DMA).
                return bass.AP(
                    tensor=xh_ap.tensor,
                    offset=(r0 + lo) * (W + 2) + shift,
                    ap=[[CHS, CIN], [1, (hi - lo) * (W + 2)]])

            def load_xband(r0, split=False):
                # split=True: two row-chunks per buffer so band-0 conv1 can
                # start as soon as the first rows land.
                chunks = [(0, 12), (12, 24), (24, RB + 4)] if split else [(0, RB + 4)]
                xa = xpool.tile([128, RB + 4, W + 2], F16, tag="xa", name="xa")
                xr = xpool.tile([128, RB + 4, W + 2], F16, tag="xr", name="xr")
                xaf = xa.rearrange("p r c -> p (r c)")
                xrf = xr.rearrange("p r c -> p (r c)")
                for lo, hi in chunks:
                    s = slice(lo * (W + 2), hi * (W + 2))
                    nc.sync.dma_start(out=xaf[0:64, s],
                                      in_=xsrc(r0, lo, hi, 0))
                    nc.sync.dma_start(out=xaf[64:128, s],
                                      in_=xsrc(r0, lo, hi, 1))
                    nc.sync.dma_start(out=xrf[0:64, s],
                                      in_=xsrc(r0, lo, hi, 0))
                    nc.sync.dma_start(out=xrf[64:128, s],
                                      in_=xsrc(r0, lo, hi, W + 2))
                return xa, xr

            # DMA issue order: band-0 x first (conv1 can start ~6us in),
            # then w1 + biases (needed with it), then the bulky w2 (needed
            # at ~55us), then logit constants (needed at ~60us).
            w1rbuf = singles.tile([128, 2, 128], F16, tag="w1rbuf")
            nc.sync.dma_start(out=w1rbuf, in_=w1r.ap())
            w1r_sb = {mc: w1rbuf[:, mc, :] for mc in range(2)}
            w1sbuf = singles.tile([128, 2, 128], F16, tag="w1sbuf")
            nc.sync.dma_start(out=w1sbuf, in_=w1s.ap())
            w1s_sb = {mc: w1sbuf[:, mc, :] for mc in range(2)}

            cstbuf = singles.tile([128, 2 * K + 4], F32, tag="cstbuf")
            nc.sync.dma_start(out=cstbuf, in_=cst.ap())

            xband0 = load_xband(0, split=True)
            bp_sb = cstbuf[:, 0:K]
            lab_sb = cstbuf[:, K:2 * K]
            b1_sb = {mc: cstbuf[:, 2 * K + mc:2 * K + mc + 1] for mc in range(2)}
            b2_sb = {mc: cstbuf[:, 2 * K + 2 + mc:2 * K + 3 + mc] for mc in range(2)}

            # conv2 weights are 1D-Winograd F(2,3) transformed along kh:
            # 4 positions x 2 kc x 3 dc x 2 mc slices of [128 cin, 128 out].
            w2buf = singles.tile([128, 48, 128], F16, tag="w2buf")
            nc.sync.dma_start(out=w2buf, in_=w2l.ap())
            gw_sb = {(pos, kc, dc, mc): w2buf[:, ((pos * 2 + kc) * 3 + dc) * 2 + mc, :]
                     for pos in range(4) for kc in range(2)
                     for dc in range(3) for mc in range(2)}

            muwbuf = singles.tile([128, 2, K], F16, tag="muwbuf")
            nc.sync.dma_start(out=muwbuf, in_=muw.ap())
            muw_sb = {kc: muwbuf[:, kc, :] for kc in range(2)}
            idnbuf = singles.tile([128, 128], F32, tag="idnbuf")
            nc.sync.dma_start(out=idnbuf, in_=idn.ap())

            # ---- bands ------------------------------------------------
            for band in range(NBAND):
                r0 = RB * band
                # x halo in two packings:
                #  xa: p0-63 = xh rows r0..r0+19, p64-127 = same shifted +1 col
                #  xr: p0-63 = xh rows,           p64-127 = same shifted +1 row
                xa, xr = xband0 if band == 0 else load_xband(r0)

                # q1 band buffer: slot j = q1 row (r0 - 1 + j), cols 1..256
                # real, cols 0/257 replicate pads.
                q1b = {}
                for mc in range(2):
                    q1b[mc] = q1pool.tile([128, RB + 2, W + 2], F16, tag=f"q1_{mc}", name=f"q1_{mc}")

                # conv1: q1 slot j needs xh local rows j+dr (pairs), and
                # taps (0,2),(1,2) from xr row j, tap (2,2) from xa row j+2.
                if band == 0:
                    groups1 = [(j, 2) for j in range(1, RB + 1, 2)] + [(RB + 1, 1)]
                else:
                    groups1 = [(j, 2) for j in range(0, RB + 2, 2)]
                for j, nr in groups1:
                    for mc in range(2):
                        ps = ps1.tile([128, nr, W], F32, tag="c1ps", name="c1ps")
                        for dr in range(3):
                            nc.tensor.matmul(
                                ps, w1a_sb[mc, dr],
                                xa[:, j + dr:j + dr + nr, 0:W],
                                start=(dr == 0), stop=False)
                        nc.tensor.matmul(ps, w1r_sb[mc],
                                         xr[:, j:j + nr, 2:W + 2],
                                         start=False, stop=False)
                        # w1s is zero-padded to 128 partitions: a 64-row
                        # matmul switches the PE into half-array mode which
                        # costs ~120ns extra turnaround per chain.
                        nc.tensor.matmul(ps, w1s_sb[mc],
                                         xa[:, j + 2:j + 2 + nr, 2:W + 2],
                                         start=False, stop=True)
                        nc.scalar.activation(
                            out=q1b[mc][:, j:j + nr, 1:W + 1], in_=ps,
                            func=AF.Relu, bias=b1_sb[mc], scale=1.0)
                # replicate pads: cols, then (band 0) top row
                for mc in range(2):
                    lo = 1 if band == 0 else 0
                    nc.vector.tensor_copy(
                        out=q1b[mc][:, lo:RB + 2, 0:1],
                        in_=q1b[mc][:, lo:RB + 2, 1:2])
                    nc.vector.tensor_copy(
                        out=q1b[mc][:, lo:RB + 2, W + 1:W + 2],
                        in_=q1b[mc][:, lo:RB + 2, W:W + 1])
                    if band == 0:
                        nc.vector.tensor_copy(
                            out=q1b[mc][:, 0:1, :], in_=q1b[mc][:, 1:2, :])

                ob = obuf.tile([128, RB // 2, 4], F32, tag="ob", name="ob")
                for g in range(RB // 2):
                    # conv2 via 1D-Winograd F(2,3) over rows: output rows
                    # (2g, 2g+1) from q1 slots 2g..2g+3 (rows 2g-1..2g+2).
                    # Forward transform (vector, fp16):
                    #   t0 = d0-d2; t1 = d1+d2; t2 = d2-d1; t3 = d1-d3
                    tb = {}
                    for kc in range(2):
                        tb[kc] = tpool.tile([128, 4, W + 2], F16,
                                            tag=f"tb{kc}", name=f"tb{kc}")
                        q = q1b[kc]
                        j = 2 * g
                        nc.vector.tensor_tensor(
                            tb[kc][:, 0], q[:, j], q[:, j + 2], ALU.subtract)
                        nc.vector.tensor_tensor(
                            tb[kc][:, 1], q[:, j + 1], q[:, j + 2], ALU.add)
                        # t2/t3 run on GpSimd (idle otherwise) to keep the
                        # vector engine off the critical path
                        nc.gpsimd.tensor_tensor(
                            tb[kc][:, 2], q[:, j + 2], q[:, j + 1], ALU.subtract)
                        nc.gpsimd.tensor_tensor(
                            tb[kc][:, 3], q[:, j + 1], q[:, j + 3], ALU.subtract)
                    # Position GEMMs: Yp = sum_{kc,dc} gw[p,kc,dc] @ t[kc][p]
                    # packed two positions per psum bank; then inverse
                    # A^T: y0 = Y0+Y1+Y2, y1 = Y1-Y2-Y3 (vector) + bias/relu
                    # (scalar).
                    q2t = {}
                    for mc in range(2):
                        pa = ps2.tile([128, 2, W], F32, tag="pA", name="pa")
                        pb = ps2.tile([128, 2, W], F32, tag="pB", name="pb")
                        for pos in range(4):
                            pt = (pa, pb)[pos // 2][:, pos % 2, :]
                            n_mm = 0
                            for kc in range(2):
                                for dc in range(3):
                                    nc.tensor.matmul(
                                        pt, gw_sb[pos, kc, dc, mc],
                                        tb[kc][:, pos, dc:dc + W],
                                        start=(n_mm == 0), stop=(n_mm == 5))
                                    n_mm += 1
                        # vector ops may read only one PSUM operand (and
                        # GpSimd cannot read PSUM at all), so Y1 goes to SBUF
                        # first via a scalar-engine copy.
                        s1 = smx.tile([128, W], F32, tag="s1", name="s1")
                        nc.scalar.activation(out=s1, in_=pa[:, 1, :],
                                             func=AF.Copy, scale=1.0)
                        vu = smx.tile([128, W], F32, tag="vu", name="vu")
                        nc.vector.tensor_tensor(vu, s1, pa[:, 0, :], ALU.add)
                        v0 = smx.tile([128, W], F32, tag="v0", name="v0")
                        nc.vector.tensor_tensor(v0, vu, pb[:, 0, :], ALU.add)
                        vw = smx.tile([128, W], F32, tag="vw", name="vw")
                        nc.vector.tensor_tensor(vw, s1, pb[:, 0, :],
                                                ALU.subtract)
                        v1 = smx.tile([128, W], F32, tag="v1", name="v1")
                        nc.vector.tensor_tensor(v1, vw, pb[:, 1, :],
                                                ALU.subtract)
                        q2t[mc] = q2pool.tile([128, 2, W], F16, tag=f"q2_{mc}", name=f"q2_{mc}")
                        nc.scalar.activation(out=q2t[mc][:, 0, :], in_=v0,
                                             func=AF.Relu, bias=b2_sb[mc],
                                             scale=1.0)
                        nc.scalar.activation(out=q2t[mc][:, 1, :], in_=v1,
                                             func=AF.Relu, bias=b2_sb[mc],
                                             scale=1.0)
                    # logits: [128 px, K] per 128-px slice, q2 stationary
                    pl = psl.tile([128, 4, K], F32, tag="lps", name="lps")
                    for j in range(4):
                        for kc in range(2):
                            q2flat = q2t[kc].rearrange("p a b -> p (a b)")
                            nc.tensor.matmul(
                                pl[:, j, :], q2flat[:, 128 * j:128 * (j + 1)],
                                muw_sb[kc], start=(kc == 0), stop=(kc == 1))
                    # softmax over K (free axis) + label contraction
                    li = smx.tile([128, 4, K], F32, tag="li", name="li")
                    nc.vector.tensor_tensor(
                        li, pl,
                        bp_sb.unsqueeze(1).to_broadcast([128, 4, K]),
                        ALU.add)
                    mx = smx.tile([128, 4], F32, tag="mx", name="mx")
                    nc.vector.reduce_max(mx, li, axis=AX.X)
                    ls = smx.tile([128, 4, K], F32, tag="ls", name="ls")
                    nc.vector.tensor_tensor(
                        ls, li,
                        mx.unsqueeze(2).to_broadcast([128, 4, K]),
                        ALU.subtract)
                    ex = smx.tile([128, 4, K], F32, tag="ex", name="ex")
                    nc.scalar.activation(out=ex, in_=ls, func=AF.Exp)
                    el = smx.tile([128, 4, K], F32, tag="el", name="el")
                    nc.vector.tensor_tensor(
                        el, ex,
                        lab_sb.unsqueeze(1).to_broadcast([128, 4, K]),
                        ALU.mult)
                    ssum = smx.tile([128, 4], F32, tag="ssum", name="ssum")
                    nc.vector.reduce_sum(ssum, ex, axis=AX.X)
                    wsum = smx.tile([128, 4], F32, tag="wsum", name="wsum")
                    nc.vector.reduce_sum(wsum, el, axis=AX.X)
                    rs = smx.tile([128, 4], F32, tag="rs", name="rs")
                    nc.vector.reciprocal(rs, ssum)
                    nc.vector.tensor_tensor(ob[:, g], wsum, rs, ALU.mult)

                # ob[p, g, (r jj)] -> PE-transpose to row-major [32, 256] so
                # the dram write is 32 contiguous 1KB rows instead of a
                # 4-byte-element scatter (which cost ~24us exposed at the
                # kernel tail).
                obr = ob.rearrange("p g (r jj) -> p g r jj", r=2)
                ot = obuf.tile([RB, W], F32, tag="ot", name="ot")
                for jj in range(2):
                    tps = psl.tile([RB, 128], F32, tag="tps", name="tps")
                    nc.tensor.transpose(tps, obr[:, :, :, jj], idnbuf)
                    nc.vector.tensor_copy(out=ot[:, 128 * jj:128 * (jj + 1)],
                                          in_=tps)
                nc.sync.dma_start(out=outd.ap()[r0:r0 + RB, :], in_=ot)

    nc.compile()
    return nc


def prep_inputs(x, w1, b1, w2, b2, w3, b3, mu, label):
    """Full inputs -> per-core in_maps."""
    w3m = w3[:, :, 0, 0]
    muW = 2.0 * (mu @ w3m)                                   # [K, Q]
    bpv = (2.0 * (mu @ b3) - (mu * mu).sum(1)).astype(np.float32)

    def pack_w(w1f, w2f):
        cinw = w1f.shape[1]
        w1a = np.empty((2, 3, 128, 128), np.float32)
        w1r = np.empty((2, 128, 128), np.float32)
        w1s = np.zeros((2, 128, 128), np.float32)
        for mc in range(2):
            ms = slice(128 * mc, 128 * (mc + 1))
            for dr in range(3):
                w1a[mc, dr, 0:64] = w1f[ms, :, dr, 0].T
                w1a[mc, dr, 64:128] = w1f[ms, :, dr, 1].T
            w1r[mc, 0:64] = w1f[ms, :, 0, 2].T
            w1r[mc, 64:128] = w1f[ms, :, 1, 2].T
            w1s[mc, 0:64] = w1f[ms, :, 2, 2].T
        # 1D-Winograd F(2,3) transform of w2 along kh:
        #   g0 = w[0]; g1 = (w[0]+w[1]+w[2])/2; g2 = (w[0]-w[1]+w[2])/2; g3 = w[2]
        w0, wk1, wk2 = w2f[:, :, 0, :], w2f[:, :, 1, :], w2f[:, :, 2, :]
        gws = [w0, 0.5 * (w0 + wk1 + wk2), 0.5 * (w0 - wk1 + wk2), wk2]
        w2p = np.empty((48, 128, 128), np.float32)
        for pos in range(4):
            for kc in range(2):
                for dc in range(3):
                    for mc in range(2):
                        idx = ((pos * 2 + kc) * 3 + dc) * 2 + mc
                        w2p[idx] = gws[pos][128 * mc:128 * (mc + 1),
                                            128 * kc:128 * (kc + 1), dc].T
        return (np.ascontiguousarray(w1a.reshape(6, 128, 128).transpose(1, 0, 2)).astype(np.float16),
                np.ascontiguousarray(w1r.transpose(1, 0, 2)).astype(np.float16),
                np.ascontiguousarray(w1s.transpose(1, 0, 2)).astype(np.float16),
                np.ascontiguousarray(w2p.transpose(1, 0, 2)).astype(np.float16))

    packs = {}
    packs[0] = pack_w(w1, w2)
    packs[1] = pack_w(w1[:, :, ::-1, :], w2[:, :, ::-1, :])

    muwp = np.empty((128, 2, K), np.float32)
    for kc in range(2):
        muwp[:, kc, :] = muW[:, 128 * kc:128 * (kc + 1)].T
    muwp = muwp.astype(np.float16)
    cstv = np.empty((128, 2 * K + 4), np.float32)
    cstv[:, 0:K] = bpv[None, :]
    cstv[:, K:2 * K] = label[None, :].astype(np.float32)
    for mc in range(2):
        cstv[:, 2 * K + mc] = b1[128 * mc:128 * (mc + 1)]
        cstv[:, 2 * K + 2 + mc] = b2[128 * mc:128 * (mc + 1)]

    rows = np.clip(np.arange(132) - 2, 0, H - 1)
    cols = np.clip(np.arange(W + 2) - 1, 0, W - 1)
    in_maps = []
    for core in range(NCORES):
        img, half = core // 2, core % 2
        xl = x[img] if half == 0 else x[img, :, ::-1, :]
        xhv = np.ascontiguousarray(xl[:, rows][:, :, cols]).astype(np.float16)
        xhf = np.concatenate([xhv.reshape(-1),
                              np.zeros(2 * (W + 2), np.float16)])
        w1ap, w1rp, w1sp, w2p = packs[half]
        in_maps.append({
            'xh': xhf, 'w1a': w1ap, 'w1r': w1rp, 'w1s': w1sp, 'w2l': w2p,
            'muw': muwp, 'cst': cstv, 'idn': np.eye(128, dtype=np.float32),
        })
    return in_maps


def gather(results, dtype=np.float32):
    out = np.empty((B, 1, H, W), dtype)
    for core in range(NCORES):
        img, half = core // 2, core % 2
        o = results[core]['out']
        if half == 0:
            out[img, 0, 0:128] = o
        else:
            out[img, 0, 128:256] = o[::-1]
    return out


def get_nc():
    if 'nc' not in _cached:
        _cached['nc'] = build_nc()
    return _cached['nc']


def kernel(x, w1, b1, w2, b2, w3, b3, mu, label, **run_kwargs):
    nc = get_nc()
    in_maps = prep_inputs(
        np.asarray(x, np.float32), np.asarray(w1, np.float32),
        np.asarray(b1, np.float32), np.asarray(w2, np.float32),
        np.asarray(b2, np.float32), np.asarray(w3, np.float32),
        np.asarray(b3, np.float32), np.asarray(mu, np.float32),
        np.asarray(label, np.float32))
    res = run_bass_kernel_spmd(nc, in_maps, core_ids=list(range(NCORES)),
                               **run_kwargs)
    out = gather(res.results)
    if run_kwargs:
        _cached['last_result'] = res
    return out

